# revision 1
# baseline (speedup 1.0000x reference)
"""CrossScaleAttention Trainium2 kernel.

Data-parallel over batch: 16 samples / 8 cores = 2 samples per core.

Algebraic restructuring (all exact up to fp reassociation):
  - 1x1 convs commute with nearest 2x upsample -> whole main branch runs at 32x32.
  - W_align folded into main qkv:  W_qkv_m @ W_align.
  - pos embeds / rel_pos / biases folded into GEMM bias vectors.
  - attention score via quadratic form:  score = x^T A x + u.x (+const dropped:
    softmax shift-invariant), A = Wq^T Wk / sqrt(C), so q/k are never computed.
  - proj GEMMs folded into fuse:  W_f* = W_fuse_half @ W_proj (+1/2 for the
    upsampled-softmax duplication on the main branch).
  - attn applied post-GEMM:  W@(attn*v) == attn*(W@v).
Matmuls run in float32r (TF32-like, ~1e-4 rel err, 4x fp32 rate at N>=512).
"""
import sys
sys.path.insert(0, '/opt/trn_rl_repo')
import numpy as np

B, CS, CM = 16, 256, 512
HS = WS = 64
HM = WM = 32
NPX_S = HS * WS          # 4096
NPX_M = HM * WM          # 1024
NCORES = 8
BPC = B // NCORES        # 2 samples per core
CHUNK = 1024             # small-branch pixel chunk (16 h-rows)
NCHUNK = NPX_S // CHUNK  # 4

_prog = None  # (nc, input_names) cache


def _build_program():
    import concourse.bacc as bacc
    import concourse.mybir as mybir
    from concourse.tile import TileContext

    f32, f32r = mybir.dt.float32, mybir.dt.float32r
    Act = mybir.ActivationFunctionType
    Alu = mybir.AluOpType

    nc = bacc.Bacc(None, target_bir_lowering=False)

    xs_d = nc.dram_tensor("xs", [BPC, CS, NPX_S], f32r, kind="ExternalInput")
    xm_d = nc.dram_tensor("xm", [BPC, CM, NPX_M], f32r, kind="ExternalInput")
    AsT_d = nc.dram_tensor("AsT", [CS, CS], f32r, kind="ExternalInput")
    WvsT_d = nc.dram_tensor("WvsT", [CS, CS], f32r, kind="ExternalInput")
    WfsT_d = nc.dram_tensor("WfsT", [CS, CS], f32r, kind="ExternalInput")
    AmT_d = nc.dram_tensor("AmT", [CM, CM], f32r, kind="ExternalInput")
    WvmT_d = nc.dram_tensor("WvmT", [CM, CS], f32r, kind="ExternalInput")
    WfmT_d = nc.dram_tensor("WfmT", [CS, CS], f32r, kind="ExternalInput")
    us_d = nc.dram_tensor("us", [128, 2], f32, kind="ExternalInput")
    cvs_d = nc.dram_tensor("cvs", [128, 2], f32, kind="ExternalInput")
    um_d = nc.dram_tensor("um", [128, 4], f32, kind="ExternalInput")
    cvm_d = nc.dram_tensor("cvm", [128, 2], f32, kind="ExternalInput")
    bout_d = nc.dram_tensor("bout", [128, 2], f32, kind="ExternalInput")
    out_d = nc.dram_tensor("out", [BPC, CS, NPX_S], f32, kind="ExternalOutput")

    with TileContext(nc) as tc:
        with (
            tc.tile_pool(name="wp", bufs=1) as wp,
            tc.tile_pool(name="mp", bufs=1) as mp,   # main-branch tiles (explicit bufs via tags)
            tc.tile_pool(name="sp", bufs=1) as sp,   # small-branch tiles
            tc.tile_pool(name="ps_y", bufs=3, space="PSUM") as ps_y,
            tc.tile_pool(name="ps_v", bufs=2, space="PSUM") as ps_v,
            tc.tile_pool(name="ps_f", bufs=2, space="PSUM") as ps_f,
            tc.tile_pool(name="ps_s", bufs=1, space="PSUM") as ps_s,
        ):
            # ---- resident weights ----
            def wtile(dram, rows, cols, name):
                ts = []
                for k in range(rows // 128):
                    t = wp.tile([128, cols], f32r, tag=f"{name}{k}")
                    nc.sync.dma_start(out=t[:], in_=dram[k * 128:(k + 1) * 128, :])
                    ts.append(t)
                return ts

            xm_all = []
            for b in range(BPC):
                xm_b = []
                for k in range(4):
                    t = mp.tile([128, NPX_M], f32r, tag="xm", bufs=6, name=f"xm{b}_{k}")
                    nc.sync.dma_start(out=t[:], in_=xm_d[b, k * 128:(k + 1) * 128, :])
                    xm_b.append(t)
                xm_all.append(xm_b)

            AmT = wtile(AmT_d, CM, CM, "AmT")
            WvmT = wtile(WvmT_d, CM, CS, "WvmT")
            WfmT = wtile(WfmT_d, CS, CS, "WfmT")
            AsT = wtile(AsT_d, CS, CS, "AsT")
            WvsT = wtile(WvsT_d, CS, CS, "WvsT")
            WfsT = wtile(WfsT_d, CS, CS, "WfsT")

            def vec(dram, cols, name):
                t = wp.tile([128, cols], f32, tag=name)
                nc.sync.dma_start(out=t[:], in_=dram[:])
                return t

            us = vec(us_d, 2, "us")
            cvs = vec(cvs_d, 2, "cvs")
            um = vec(um_d, 4, "um")
            cvm = vec(cvm_d, 2, "cvm")
            bout = vec(bout_d, 2, "bout")

            ones32 = wp.tile([128, 1], f32, tag="ones32")
            nc.vector.memset(ones32[:], 1.0)
            ones = wp.tile([128, 1], f32r, tag="ones")
            nc.vector.tensor_copy(ones[:], ones32[:])

            smalls = []
            for b in range(BPC):
                # ================= main branch (at 32x32) =================
                xm = xm_all[b]

                # y' = A_m @ x + u_m ; t_m = x * y'
                tm = [mp.tile([128, NPX_M], f32r, tag=f"tm{m}", bufs=2, name=f"tm{m}") for m in range(4)]
                for n in range(2):
                    for m in range(4):
                        py = ps_y.tile([128, 512], f32, tag="y")
                        for k in range(4):
                            nc.tensor.matmul(py[:], AmT[k][:, m * 128:(m + 1) * 128],
                                             xm[k][:, n * 512:(n + 1) * 512],
                                             start=(k == 0), stop=(k == 3))
                        nc.vector.scalar_tensor_tensor(
                            tm[m][:, n * 512:(n + 1) * 512], py[:], um[:, m:m + 1],
                            xm[m][:, n * 512:(n + 1) * 512], op0=Alu.add, op1=Alu.mult)

                # v_m = Wvm @ x + cvm  (f32r)
                vm = [mp.tile([128, NPX_M], f32r, tag=f"vm{m}", bufs=1, name=f"vm{m}") for m in range(2)]
                for n in range(2):
                    for m in range(2):
                        pv = ps_v.tile([128, 512], f32, tag="v")
                        for k in range(4):
                            nc.tensor.matmul(pv[:], WvmT[k][:, m * 128:(m + 1) * 128],
                                             xm[k][:, n * 512:(n + 1) * 512],
                                             start=(k == 0), stop=(k == 3))
                        nc.scalar.activation(vm[m][:, n * 512:(n + 1) * 512], pv[:],
                                             Act.Identity, bias=cvm[:, m:m + 1], scale=1.0)

                # score = sum_c t_m -> [1, 1024]
                smf = mp.tile([1, NPX_M], f32, tag="smf", bufs=1)
                for n in range(2):
                    pscr = ps_s.tile([1, 512], f32, tag="s")
                    for k in range(4):
                        nc.tensor.matmul(pscr[:], ones[:], tm[k][:, n * 512:(n + 1) * 512],
                                         start=(k == 0), stop=(k == 3))
                    nc.scalar.activation(smf[:, n * 512:(n + 1) * 512], pscr[:], Act.Copy)

                # softmax over w (32-wide rows)
                shw = mp.tile([32, 32], f32, tag="shw", bufs=2)
                nc.sync.dma_start(out=shw[:], in_=smf[:])
                nmax = mp.tile([32, 1], f32, tag="nmax", bufs=2)
                nc.vector.tensor_reduce(nmax[:], shw[:], axis=mybir.AxisListType.X,
                                        op=Alu.max, negate=True)
                ex = mp.tile([32, 32], f32, tag="ex", bufs=2)
                esum = mp.tile([32, 1], f32, tag="esum", bufs=2)
                nc.scalar.activation(ex[:], shw[:], Act.Exp, bias=nmax[:], scale=1.0,
                                     accum_out=esum[:])
                rec = mp.tile([32, 1], f32, tag="rec", bufs=2)
                nc.vector.reciprocal(rec[:], esum[:])
                attnm = mp.tile([32, 32], f32, tag="attnm", bufs=2)
                nc.scalar.activation(attnm[:], ex[:], Act.Copy, scale=rec[:])
                amf = mp.tile([1, NPX_M], f32, tag="amf", bufs=1)
                nc.sync.dma_start(out=amf[:], in_=attnm[:])
                amb = mp.tile([128, NPX_M], f32, tag="amb", bufs=2)
                nc.gpsimd.partition_broadcast(amb[:], amf[:])

                # fm = attn_m * (W_fm @ v_m)   [256,1024] f32, resident for small branch
                fm = [mp.tile([128, NPX_M], f32, tag=f"fm{m}", bufs=2, name=f"fm{m}") for m in range(2)]
                for n in range(2):
                    for m in range(2):
                        pf = ps_f.tile([128, 512], f32, tag="f")
                        for k in range(2):
                            nc.tensor.matmul(pf[:], WfmT[k][:, m * 128:(m + 1) * 128],
                                             vm[k][:, n * 512:(n + 1) * 512],
                                             start=(k == 0), stop=(k == 1))
                        nc.vector.tensor_mul(fm[m][:, n * 512:(n + 1) * 512], pf[:],
                                             amb[:, n * 512:(n + 1) * 512])

                smalls.append((b, fm))

            # ================= small branches, chunk-interleaved across samples ========
            for c in range(NCHUNK):
                for b, fm in smalls:
                    px0 = c * CHUNK
                    xs = []
                    for k in range(2):
                        t = sp.tile([128, CHUNK], f32r, tag="xs", bufs=4)
                        nc.sync.dma_start(out=t[:],
                                          in_=xs_d[b, k * 128:(k + 1) * 128, px0:px0 + CHUNK])
                        xs.append(t)

                    ts_ = [sp.tile([128, CHUNK], f32r, tag=f"ts{m}", bufs=2, name=f"ts{m}") for m in range(2)]
                    for n in range(2):
                        for m in range(2):
                            py = ps_y.tile([128, 512], f32, tag="y")
                            for k in range(2):
                                nc.tensor.matmul(py[:], AsT[k][:, m * 128:(m + 1) * 128],
                                                 xs[k][:, n * 512:(n + 1) * 512],
                                                 start=(k == 0), stop=(k == 1))
                            nc.vector.scalar_tensor_tensor(
                                ts_[m][:, n * 512:(n + 1) * 512], py[:], us[:, m:m + 1],
                                xs[m][:, n * 512:(n + 1) * 512], op0=Alu.add, op1=Alu.mult)

                    vs = [sp.tile([128, CHUNK], f32r, tag=f"vs{m}", bufs=2, name=f"vs{m}") for m in range(2)]
                    for n in range(2):
                        for m in range(2):
                            pv = ps_v.tile([128, 512], f32, tag="v")
                            for k in range(2):
                                nc.tensor.matmul(pv[:], WvsT[k][:, m * 128:(m + 1) * 128],
                                                 xs[k][:, n * 512:(n + 1) * 512],
                                                 start=(k == 0), stop=(k == 1))
                            nc.scalar.activation(vs[m][:, n * 512:(n + 1) * 512], pv[:],
                                                 Act.Identity, bias=cvs[:, m:m + 1], scale=1.0)

                    sf = sp.tile([1, CHUNK], f32, tag="sf", bufs=2)
                    for n in range(2):
                        pscr = ps_s.tile([1, 512], f32, tag="s")
                        for k in range(2):
                            nc.tensor.matmul(pscr[:], ones[:],
                                             ts_[k][:, n * 512:(n + 1) * 512],
                                             start=(k == 0), stop=(k == 1))
                        nc.scalar.activation(sf[:, n * 512:(n + 1) * 512], pscr[:], Act.Copy)

                    shw_s = sp.tile([16, 64], f32, tag="shw_s", bufs=2)
                    nc.sync.dma_start(out=shw_s[:], in_=sf[:])
                    nmax_s = sp.tile([16, 1], f32, tag="nmax_s", bufs=2)
                    nc.vector.tensor_reduce(nmax_s[:], shw_s[:], axis=mybir.AxisListType.X,
                                            op=Alu.max, negate=True)
                    ex_s = sp.tile([16, 64], f32, tag="ex_s", bufs=2)
                    esum_s = sp.tile([16, 1], f32, tag="esum_s", bufs=2)
                    nc.scalar.activation(ex_s[:], shw_s[:], Act.Exp, bias=nmax_s[:],
                                         scale=1.0, accum_out=esum_s[:])
                    rec_s = sp.tile([16, 1], f32, tag="rec_s", bufs=2)
                    nc.vector.reciprocal(rec_s[:], esum_s[:])
                    attn_s = sp.tile([16, 64], f32, tag="attn_s", bufs=2)
                    nc.scalar.activation(attn_s[:], ex_s[:], Act.Copy, scale=rec_s[:])
                    af = sp.tile([1, CHUNK], f32, tag="af", bufs=2)
                    nc.sync.dma_start(out=af[:], in_=attn_s[:])
                    ab = sp.tile([128, CHUNK], f32, tag="ab", bufs=2)
                    nc.gpsimd.partition_broadcast(ab[:], af[:])

                    fs = [sp.tile([128, CHUNK], f32, tag=f"fs{m}", bufs=2, name=f"fs{m}") for m in range(2)]
                    for n in range(2):
                        for m in range(2):
                            pf = ps_f.tile([128, 512], f32, tag="f")
                            for k in range(2):
                                nc.tensor.matmul(pf[:], WfsT[k][:, m * 128:(m + 1) * 128],
                                                 vs[k][:, n * 512:(n + 1) * 512],
                                                 start=(k == 0), stop=(k == 1))
                            nc.vector.tensor_mul(fs[m][:, n * 512:(n + 1) * 512], pf[:],
                                                 ab[:, n * 512:(n + 1) * 512])

                    # out = fs + bout + upsample(fm)   (4 strided adds per m)
                    h2a, h2b = c * 8, c * 8 + 8
                    for m in range(2):
                        f5 = fs[m][:].rearrange("p (h2 dh wj dw) -> p h2 dh wj dw",
                                                h2=8, dh=2, wj=32, dw=2)
                        fm3 = fm[m][:].rearrange("p (h2 wj) -> p h2 wj", wj=32)
                        for dh in range(2):
                            for dw in range(2):
                                nc.vector.scalar_tensor_tensor(
                                    f5[:, :, dh, :, dw], f5[:, :, dh, :, dw],
                                    bout[:, m:m + 1], fm3[:, h2a:h2b, :],
                                    op0=Alu.add, op1=Alu.add)
                        nc.sync.dma_start(
                            out=out_d[b, m * 128:(m + 1) * 128, px0:px0 + CHUNK],
                            in_=fs[m][:])

    nc.compile()
    return nc


def _prep_weights(W_align, b_align, pos_embed_main, pos_embed_small,
                  W_qkv_s, b_qkv_s, W_proj_s, b_proj_s, rel_pos_s,
                  W_qkv_m, b_qkv_m, W_proj_m, b_proj_m, rel_pos_m,
                  W_fuse, b_fuse):
    d = np.float64
    W_align, b_align = W_align.astype(d), b_align.astype(d)
    pos_s = pos_embed_small.reshape(-1).astype(d)
    pos_m = pos_embed_main.reshape(-1).astype(d)
    rel_s = rel_pos_s.reshape(-1).astype(d)
    rel_m = rel_pos_m.reshape(-1).astype(d)
    W_qkv_s, b_qkv_s = W_qkv_s.astype(d), b_qkv_s.astype(d)
    W_qkv_m, b_qkv_m = W_qkv_m.astype(d), b_qkv_m.astype(d)
    W_proj_s, b_proj_s = W_proj_s.astype(d), b_proj_s.astype(d)
    W_proj_m, b_proj_m = W_proj_m.astype(d), b_proj_m.astype(d)
    W_fuse, b_fuse = W_fuse.astype(d), b_fuse.astype(d)
    scale = 1.0 / np.sqrt(np.float64(CS))

    # small branch
    Wq, Wk, Wv = W_qkv_s[:CS], W_qkv_s[CS:2 * CS], W_qkv_s[2 * CS:]
    bq, bk, bv = b_qkv_s[:CS], b_qkv_s[CS:2 * CS], b_qkv_s[2 * CS:]
    cq = Wq @ pos_s + bq + rel_s
    ck = Wk @ pos_s + bk + rel_s
    cv_s = Wv @ pos_s + bv
    A_s = (Wq.T @ Wk) * scale
    u_s = (Wk.T @ cq + Wq.T @ ck) * scale
    Wv_s = Wv

    # main branch (W_align folded; runs at 32x32)
    cbase = b_align + pos_m
    Wqm = W_qkv_m[:CS] @ W_align
    Wkm = W_qkv_m[CS:2 * CS] @ W_align
    Wvm = W_qkv_m[2 * CS:] @ W_align
    cqm = W_qkv_m[:CS] @ cbase + b_qkv_m[:CS] + rel_m
    ckm = W_qkv_m[CS:2 * CS] @ cbase + b_qkv_m[CS:2 * CS] + rel_m
    cv_m = W_qkv_m[2 * CS:] @ cbase + b_qkv_m[2 * CS:]
    A_m = (Wqm.T @ Wkm) * scale
    u_m = (Wkm.T @ cqm + Wqm.T @ ckm) * scale

    # fuse folding (1/2 on main: upsampled softmax over 64 = low-res softmax / 2)
    W_fs = W_fuse[:, :CS] @ W_proj_s
    W_fm = (W_fuse[:, CS:] @ W_proj_m) * 0.5
    b_out = b_fuse + W_fuse[:, :CS] @ b_proj_s + W_fuse[:, CS:] @ b_proj_m

    f = np.float32
    def colvec(v, chunks):
        return np.ascontiguousarray(v.reshape(chunks, 128).T.astype(f))
    return {
        "AsT": np.ascontiguousarray(A_s.T.astype(f)),
        "WvsT": np.ascontiguousarray(Wv_s.T.astype(f)),
        "WfsT": np.ascontiguousarray(W_fs.T.astype(f)),
        "AmT": np.ascontiguousarray(A_m.T.astype(f)),
        "WvmT": np.ascontiguousarray(Wvm.T.astype(f)),
        "WfmT": np.ascontiguousarray(W_fm.T.astype(f)),
        "us": colvec(u_s, 2), "cvs": colvec(cv_s, 2),
        "um": colvec(u_m, 4), "cvm": colvec(cv_m, 2),
        "bout": colvec(b_out, 2),
    }


def kernel(**inputs):
    global _prog
    from concourse.bass_utils import run_bass_kernel_spmd

    small = np.ascontiguousarray(np.asarray(inputs["small_feat"], dtype=np.float32))
    main = np.ascontiguousarray(np.asarray(inputs["main_feat"], dtype=np.float32))
    w = _prep_weights(**{k: np.asarray(v) for k, v in inputs.items()
                         if k not in ("small_feat", "main_feat")})

    if _prog is None:
        _prog = _build_program()
    nc = _prog

    in_maps = []
    for c in range(NCORES):
        m = dict(w)
        m["xs"] = small[c * BPC:(c + 1) * BPC].reshape(BPC, CS, NPX_S)
        m["xm"] = main[c * BPC:(c + 1) * BPC].reshape(BPC, CM, NPX_M)
        in_maps.append(m)

    res = run_bass_kernel_spmd(nc, in_maps, list(range(NCORES)))
    out = np.concatenate([r["out"] for r in res.results], axis=0)
    return out.reshape(B, CS, HS, WS)



# revision 2
# speedup vs baseline: 1.0813x; 1.0813x over previous
"""CrossScaleAttention Trainium2 kernel, v3 (pipelined schedule).

Data-parallel over batch: 16 samples / 8 cores = 2 per core.

Math (per sample), exact up to quantization:
  small: y = As8 @ xs8 (fp8 DoubleRow, A scaled by 256)
         t16 = (y + us') * xse16          (us' = (u - A^T e_s)*256)
         score = sum_c t16 / 256          (fp16 ones-matmul reduce)
         exp fused into the PSUM exit (Act), row-sums/recip on DVE
         out_psum = Wc16 @ (attn*xse16) + I @ up(fm) ; out = fp16(psum + b_out)
  main:  same quadratic-form score at 32x32 (Am fp8-DR over 512 ch),
         fm = Wcm16 @ (attn_m*xme16)  (0.5 folded into Wcm host-side),
         fm w-duplicated into fm2 [128,2048] for the upsample matmul.
  Folds: W_align into main weights; attn*cc terms into inputs via
         e = solve(Wc, cc); proj+fuse into Wc; u adjusted by A^T e.
"""
import sys
sys.path.insert(0, '/opt/trn_rl_repo')
import numpy as np
import ml_dtypes

B, CS, CM = 16, 256, 512
HS = WS = 64
HM = WM = 32
NPX_S = HS * WS          # 4096
NPX_M = HM * WM          # 1024
NCORES = 8
BPC = B // NCORES        # 2
SCL = 256.0
F8 = ml_dtypes.float8_e4m3
NH = 2                   # small-branch halves per sample
HPX = NPX_S // NH        # 2048

_prog = None


def _build_program():
    import concourse.bacc as bacc
    import concourse.mybir as mybir
    from concourse.tile import TileContext

    f32, f16, f8 = mybir.dt.float32, mybir.dt.float16, mybir.dt.float8e4
    Act = mybir.ActivationFunctionType
    Alu = mybir.AluOpType
    DR = mybir.MatmulPerfMode.DoubleRow

    nc = bacc.Bacc(None, target_bir_lowering=False)

    xs8_d = nc.dram_tensor("xs8", [BPC, 128, 2, NPX_S], f8, kind="ExternalInput")
    xse_d = nc.dram_tensor("xse", [BPC, 2, 128, NPX_S], f16, kind="ExternalInput")
    xm8_d = nc.dram_tensor("xm8", [BPC, 128, 4, NPX_M], f8, kind="ExternalInput")
    xme_d = nc.dram_tensor("xme", [BPC, 128, 4, NPX_M], f16, kind="ExternalInput")
    As8_d = nc.dram_tensor("As8", [128, 2, 256], f8, kind="ExternalInput")
    Am8_d = nc.dram_tensor("Am8", [128, 4, 512], f8, kind="ExternalInput")
    WcT_d = nc.dram_tensor("WcT", [128, 2, 256], f16, kind="ExternalInput")
    WmT_d = nc.dram_tensor("WmT", [128, 4, 256], f16, kind="ExternalInput")
    I16_d = nc.dram_tensor("I16", [128, 128], f16, kind="ExternalInput")
    us_d = nc.dram_tensor("usv", [128, 2], f32, kind="ExternalInput")
    um_d = nc.dram_tensor("umv", [128, 4], f32, kind="ExternalInput")
    bo_d = nc.dram_tensor("bov", [128, 2], f32, kind="ExternalInput")
    out_d = nc.dram_tensor("out", [BPC, 2, 128, NPX_S], f16, kind="ExternalOutput")

    with TileContext(nc) as tc:
        with (
            tc.tile_pool(name="wp", bufs=1) as wp,
            tc.tile_pool(name="xp", bufs=1) as xp,
            tc.tile_pool(name="tp", bufs=1) as tp,
            tc.tile_pool(name="ps_y", bufs=2, space="PSUM") as ps_y,
            tc.tile_pool(name="ps_s", bufs=1, space="PSUM") as ps_s,
            tc.tile_pool(name="ps_v", bufs=2, space="PSUM") as ps_v,
        ):
            st = {}

            def dma_w(name, dram, shape, dtype):
                t = wp.tile(shape, dtype, tag=name, name=name)
                nc.sync.dma_start(out=t[:], in_=dram[:])
                return t

            # DMA emission order = availability order for the pipeline
            As8 = dma_w("As8", As8_d, [128, 2, 256], f8)
            usv = dma_w("usv", us_d, [128, 2], f32)
            xm8, xme = [None] * BPC, [None] * BPC

            def load_xm(b):
                t = xp.tile([128, 4, NPX_M], f8, tag=f"xm8_{b}", name=f"xm8_{b}")
                nc.sync.dma_start(out=t[:], in_=xm8_d[b])
                xm8[b] = t
                tk = xp.tile([128, 4, NPX_M], f16, tag=f"xme_{b}", name=f"xme_{b}")
                nc.sync.dma_start(out=tk[:], in_=xme_d[b])
                xme[b] = tk

            xs8, xse = [None] * BPC, [None] * BPC

            def alloc_xs(b):
                t = xp.tile([128, 2, NPX_S], f8, tag=f"xs8_{b}", name=f"xs8_{b}")
                xs8[b] = t
                blocks = []
                for k in range(2):
                    tk = xp.tile([128, NPX_S], f16, tag=f"xse_{b}_{k}", name=f"xse_{b}_{k}")
                    blocks.append(tk)
                xse[b] = blocks

            def load_xs_half(b, h):
                nc.sync.dma_start(out=xs8[b][:, :, h * HPX:(h + 1) * HPX],
                                  in_=xs8_d[b, :, :, h * HPX:(h + 1) * HPX])
                for k in range(2):
                    nc.sync.dma_start(out=xse[b][k][:, h * HPX:(h + 1) * HPX],
                                      in_=xse_d[b, k, :, h * HPX:(h + 1) * HPX])

            alloc_xs(0)
            alloc_xs(1)
            load_xs_half(0, 0)
            Am8 = dma_w("Am8", Am8_d, [128, 4, 512], f8)
            umv = dma_w("umv", um_d, [128, 4], f32)
            load_xm(0)
            load_xs_half(0, 1)
            WmT = dma_w("WmT", WmT_d, [128, 4, 256], f16)
            WcT = dma_w("WcT", WcT_d, [128, 2, 256], f16)
            I16 = dma_w("I16", I16_d, [128, 128], f16)
            bov = dma_w("bov", bo_d, [128, 2], f32)
            load_xs_half(1, 0)
            load_xm(1)
            load_xs_half(1, 1)

            ones16 = wp.tile([128, 1], f16, tag="ones16")
            nc.vector.memset(ones16[:], 1.0)

            # ---------------- unit emitters ----------------
            def ms_gemm(b):
                tm16 = tp.tile([128, 4, NPX_M], f16, tag="tm16", bufs=2, name=f"tm16_{b}")
                for m in range(4):
                    ym = ps_y.tile([128, NPX_M], f32, tag="y1k")
                    for nj in range(2):
                        for kk in range(2):
                            nc.tensor.matmul(
                                ym[:, nj * 512:(nj + 1) * 512],
                                Am8[:, 2 * kk:2 * kk + 2, m * 128:(m + 1) * 128],
                                xm8[b][:, 2 * kk:2 * kk + 2, nj * 512:(nj + 1) * 512],
                                start=(kk == 0), stop=(kk == 1), perf_mode=DR)
                    nc.vector.scalar_tensor_tensor(
                        tm16[:, m, :], ym[:], umv[:, m:m + 1], xme[b][:, m, :],
                        op0=Alu.add, op1=Alu.mult)
                em = tp.tile([32, 32], f16, tag="em32", bufs=2, name=f"em32_{b}")
                sps = ps_s.tile([1, 1024], f32, tag="s")
                for nj in range(2):
                    for k in range(4):
                        nc.tensor.matmul(sps[:, nj * 512:(nj + 1) * 512], ones16[:],
                                         tm16[:, k, nj * 512:(nj + 1) * 512],
                                         start=(k == 0), stop=(k == 3))
                ef = tp.tile([1, 1024], f16, tag="efm", bufs=2)
                nc.scalar.activation(ef[:], sps[:], Act.Exp, scale=1.0 / SCL)
                nc.sync.dma_start(out=em[:], in_=ef[:])
                st[("em", b)] = em

            def ms_tail(b, eng="dve"):
                em = st[("em", b)]
                esum = tp.tile([32, 1], f32, tag="esm", bufs=2)
                nc.vector.tensor_reduce(esum[:], em[:], axis=mybir.AxisListType.X,
                                        op=Alu.add)
                rec = tp.tile([32, 1], f32, tag="recm", bufs=2)
                nc.vector.reciprocal(rec[:], esum[:])
                at16 = tp.tile([32, 32], f16, tag="atm", bufs=2)
                nc.vector.tensor_scalar_mul(at16[:], em[:], rec[:])
                amf = tp.tile([1, NPX_M], f16, tag="amf", bufs=2)
                nc.sync.dma_start(out=amf[:], in_=at16[:])
                abm = tp.tile([128, NPX_M], f16, tag="abm", bufs=2)
                nc.gpsimd.partition_broadcast(abm[:], amf[:])
                axm = tp.tile([128, 4, NPX_M], f16, tag="axm", bufs=2, name=f"axm_{b}")
                engf = nc.vector if eng == "dve" else nc.gpsimd
                for k in range(4):
                    engf.tensor_tensor(axm[:, k, :], abm[:], xme[b][:, k, :],
                                       op=Alu.mult)
                st[("axm", b)] = axm

            def mv(b):
                axm = st[("axm", b)]
                fm2_b = []
                for m in range(2):
                    f2 = tp.tile([128, 2048], f16, tag=f"fm2_{b}_{m}", bufs=1,
                                 name=f"fm2_{b}_{m}")
                    f2v = f2[:].rearrange("p (w d) -> p w d", d=2)
                    for nj in range(2):
                        fv = ps_v.tile([128, 512], f32, tag="v")
                        for k in range(4):
                            nc.tensor.matmul(fv[:],
                                             WmT[:, k, m * 128:(m + 1) * 128],
                                             axm[:, k, nj * 512:(nj + 1) * 512],
                                             start=(k == 0), stop=(k == 3))
                        for d in range(2):
                            nc.scalar.activation(f2v[:, nj * 512:(nj + 1) * 512, d],
                                                 fv[:], Act.Identity)
                    fm2_b.append(f2)
                st[("fm2", b)] = fm2_b

            def ss_gemm(b, h):
                for c in range(2):
                    cc = h * 2 + c
                    sc = tp.tile([16, 64], f16, tag="sc64", bufs=4)
                    px0 = cc * 1024
                    t16h = tp.tile([128, 2, 1024], f16, tag="t16h", bufs=3)
                    for m in range(2):
                        yps = ps_y.tile([128, 1024], f32, tag="y1k")
                        for nj in range(2):
                            nc.tensor.matmul(
                                yps[:, nj * 512:(nj + 1) * 512],
                                As8[:, :, m * 128:(m + 1) * 128],
                                xs8[b][:, :, px0 + nj * 512:px0 + (nj + 1) * 512],
                                start=True, stop=True, perf_mode=DR)
                        nc.vector.scalar_tensor_tensor(
                            t16h[:, m, :], yps[:], usv[:, m:m + 1],
                            xse[b][m][:, px0:px0 + 1024],
                            op0=Alu.add, op1=Alu.mult)
                    sps = ps_s.tile([1, 1024], f32, tag="s")
                    for nj in range(2):
                        for k in range(2):
                            nc.tensor.matmul(sps[:, nj * 512:(nj + 1) * 512], ones16[:],
                                             t16h[:, k, nj * 512:(nj + 1) * 512],
                                             start=(k == 0), stop=(k == 1))
                    ef = tp.tile([1, 1024], f16, tag="efs", bufs=3)
                    nc.scalar.activation(ef[:], sps[:], Act.Exp, scale=1.0 / SCL)
                    nc.sync.dma_start(out=sc[:], in_=ef[:])
                    st[("sc", b, cc)] = sc

            def ss_tail(b, h, eng="dve"):
                engf = nc.vector if eng == "dve" else nc.gpsimd
                for c in range(2):
                    cc = h * 2 + c
                    px0 = cc * 1024
                    sc = st[("sc", b, cc)]
                    esum = tp.tile([16, 1], f32, tag="ess", bufs=3)
                    nc.vector.tensor_reduce(esum[:], sc[:], axis=mybir.AxisListType.X,
                                            op=Alu.add)
                    rec = tp.tile([16, 1], f32, tag="recs", bufs=3)
                    nc.vector.reciprocal(rec[:], esum[:])
                    at16 = tp.tile([16, 64], f16, tag="ats", bufs=3)
                    nc.vector.tensor_scalar_mul(at16[:], sc[:], rec[:])
                    af = tp.tile([1, 1024], f16, tag="afs", bufs=3)
                    nc.sync.dma_start(out=af[:], in_=at16[:])
                    ab = tp.tile([128, 1024], f16, tag="abs", bufs=3)
                    nc.gpsimd.partition_broadcast(ab[:], af[:])
                    axs = tp.tile([128, 2, 1024], f16, tag="axs", bufs=5)
                    for k in range(2):
                        engf.tensor_tensor(axs[:, k, :], ab[:],
                                           xse[b][k][:, px0:px0 + 1024],
                                           op=Alu.mult)
                    st[("axs", b, cc)] = axs

            def sv(b, h, out_eng="act"):
                fm2_b = st[("fm2", b)]
                for m in range(2):
                    fmv = fm2_b[m][:].rearrange("p (hh one wd) -> p hh one wd",
                                                one=1, wd=64)
                    for nj in range(4):
                        px0 = h * HPX + nj * 512
                        axs = st[("axs", b, h * 2 + nj // 2)]
                        vps = ps_v.tile([128, 512], f32, tag="v")
                        for k in range(2):
                            nc.tensor.matmul(
                                vps[:], WcT[:, k, m * 128:(m + 1) * 128],
                                axs[:, k, (nj % 2) * 512:(nj % 2 + 1) * 512],
                                start=(k == 0), stop=False)
                        h0 = (h * 4 + nj) * 4
                        up_ap = fmv[:, h0:h0 + 4].broadcast_to([128, 4, 2, 64])
                        nc.tensor.matmul(vps[:], I16[:], up_ap, start=False, stop=True)
                        if nj == 0:
                            o16 = tp.tile([128, 2048], f16, tag="o16", bufs=4)
                        if out_eng == "act":
                            nc.scalar.activation(o16[:, nj * 512:(nj + 1) * 512],
                                                 vps[:], Act.Identity,
                                                 bias=bov[:, m:m + 1], scale=1.0)
                        else:
                            nc.vector.tensor_scalar_add(o16[:, nj * 512:(nj + 1) * 512],
                                                        vps[:], bov[:, m:m + 1])
                    nc.sync.dma_start(out=out_d[b, m][:, h * HPX:(h + 1) * HPX],
                                      in_=o16[:])

            # ---------------- pipelined schedule ----------------
            ss_gemm(0, 0)
            ms_gemm(0)
            ss_tail(0, 0)
            ms_tail(0)
            ss_gemm(0, 1)
            mv(0)
            ss_tail(0, 1)
            sv(0, 0)
            ss_gemm(1, 0)
            sv(0, 1)
            ss_gemm(1, 1)
            ms_gemm(1)
            ss_tail(1, 0)
            ms_tail(1)
            mv(1)
            sv(1, 0)
            ss_tail(1, 1)
            sv(1, 1, out_eng="dve")

    nc.compile()
    return nc


def _prep_weights(W_align, b_align, pos_embed_main, pos_embed_small,
                  W_qkv_s, b_qkv_s, W_proj_s, b_proj_s, rel_pos_s,
                  W_qkv_m, b_qkv_m, W_proj_m, b_proj_m, rel_pos_m,
                  W_fuse, b_fuse):
    d = np.float64
    g = dict(W_align=W_align, b_align=b_align, W_qkv_s=W_qkv_s, b_qkv_s=b_qkv_s,
             W_proj_s=W_proj_s, b_proj_s=b_proj_s, W_qkv_m=W_qkv_m, b_qkv_m=b_qkv_m,
             W_proj_m=W_proj_m, b_proj_m=b_proj_m, W_fuse=W_fuse, b_fuse=b_fuse)
    g = {k: np.asarray(v, d) for k, v in g.items()}
    pos_s = np.asarray(pos_embed_small, d).reshape(-1)
    pos_m = np.asarray(pos_embed_main, d).reshape(-1)
    rel_s = np.asarray(rel_pos_s, d).reshape(-1)
    rel_m = np.asarray(rel_pos_m, d).reshape(-1)
    scale = 1.0 / np.sqrt(np.float64(CS))

    Wq, Wk, Wv = g['W_qkv_s'][:CS], g['W_qkv_s'][CS:2 * CS], g['W_qkv_s'][2 * CS:]
    bq, bk, bv = g['b_qkv_s'][:CS], g['b_qkv_s'][CS:2 * CS], g['b_qkv_s'][2 * CS:]
    cq = Wq @ pos_s + bq + rel_s
    ck = Wk @ pos_s + bk + rel_s
    cv_s = Wv @ pos_s + bv
    A_s = (Wq.T @ Wk) * scale
    u_s = (Wk.T @ cq + Wq.T @ ck) * scale

    cbase = g['b_align'] + pos_m
    Wqm = g['W_qkv_m'][:CS] @ g['W_align']
    Wkm = g['W_qkv_m'][CS:2 * CS] @ g['W_align']
    Wvm = g['W_qkv_m'][2 * CS:] @ g['W_align']
    cqm = g['W_qkv_m'][:CS] @ cbase + g['b_qkv_m'][:CS] + rel_m
    ckm = g['W_qkv_m'][CS:2 * CS] @ cbase + g['b_qkv_m'][CS:2 * CS] + rel_m
    cv_m = g['W_qkv_m'][2 * CS:] @ cbase + g['b_qkv_m'][2 * CS:]
    A_m = (Wqm.T @ Wkm) * scale
    u_m = (Wkm.T @ cqm + Wqm.T @ ckm) * scale

    W_fs = g['W_fuse'][:, :CS] @ g['W_proj_s']
    W_fm = (g['W_fuse'][:, CS:] @ g['W_proj_m']) * 0.5   # upsampled-softmax halving
    b_out = g['b_fuse'] + g['W_fuse'][:, :CS] @ g['b_proj_s'] \
        + g['W_fuse'][:, CS:] @ g['b_proj_m']
    Wc_s = W_fs @ Wv
    cc_s = W_fs @ cv_s
    Wc_m = W_fm @ Wvm
    cc_m = W_fm @ cv_m
    e_s = np.linalg.solve(Wc_s, cc_s)
    e_m = np.linalg.lstsq(Wc_m, cc_m, rcond=None)[0]
    us_adj = (u_s - (A_s.T @ e_s)) * SCL
    um_adj = (u_m - (A_m.T @ e_m)) * SCL

    f = np.float32

    def q8(x):
        return np.clip(np.asarray(x, f), -240, 240).astype(F8)

    def colvec(v, chunks):
        return np.ascontiguousarray(np.asarray(v, f).reshape(chunks, 128).T)

    As8 = q8((A_s * SCL).T.reshape(2, 128, 256).transpose(1, 0, 2))
    Am8 = q8((A_m * SCL).T.reshape(4, 128, 512).transpose(1, 0, 2))
    WcT = np.ascontiguousarray(Wc_s.T.reshape(2, 128, 256).transpose(1, 0, 2)
                               .astype(np.float16))
    WmT = np.ascontiguousarray(Wc_m.T.reshape(4, 128, 256).transpose(1, 0, 2)
                               .astype(np.float16))
    return {
        "As8": np.ascontiguousarray(As8),
        "Am8": np.ascontiguousarray(Am8),
        "WcT": WcT, "WmT": WmT,
        "I16": np.eye(128, dtype=np.float16),
        "usv": colvec(us_adj, 2), "umv": colvec(um_adj, 4),
        "bov": colvec(b_out, 2),
    }, e_s.astype(f), e_m.astype(f)


def kernel(**inputs):
    global _prog
    from concourse.bass_utils import run_bass_kernel_spmd

    small = np.ascontiguousarray(np.asarray(inputs["small_feat"], dtype=np.float32))
    main = np.ascontiguousarray(np.asarray(inputs["main_feat"], dtype=np.float32))
    w, e_s, e_m = _prep_weights(**{k: np.asarray(v) for k, v in inputs.items()
                                   if k not in ("small_feat", "main_feat")})

    xs = small.reshape(B, CS, NPX_S)
    xm = main.reshape(B, CM, NPX_M)
    xs8 = np.clip(xs, -240, 240).astype(F8).reshape(B, 2, 128, NPX_S).transpose(0, 2, 1, 3)
    xse = (xs + e_s[None, :, None]).astype(np.float16).reshape(B, 2, 128, NPX_S)
    xm8 = np.clip(xm, -240, 240).astype(F8).reshape(B, 4, 128, NPX_M).transpose(0, 2, 1, 3)
    xme = (xm + e_m[None, :, None]).astype(np.float16).reshape(B, 4, 128, NPX_M) \
        .transpose(0, 2, 1, 3)

    if _prog is None:
        _prog = _build_program()
    nc = _prog

    in_maps = []
    for c in range(NCORES):
        m = dict(w)
        sl = slice(c * BPC, (c + 1) * BPC)
        m["xs8"] = np.ascontiguousarray(xs8[sl])
        m["xse"] = np.ascontiguousarray(xse[sl])
        m["xm8"] = np.ascontiguousarray(xm8[sl])
        m["xme"] = np.ascontiguousarray(xme[sl])

        in_maps.append(m)

    res = run_bass_kernel_spmd(nc, in_maps, list(range(NCORES)))
    out = np.concatenate([r["out"] for r in res.results], axis=0)
    return out.astype(np.float32).reshape(B, CS, HS, WS)


# revision 3
# speedup vs baseline: 1.0861x; 1.0044x over previous
"""CrossScaleAttention Trainium2 kernel, v3 (pipelined schedule).

Data-parallel over batch: 16 samples / 8 cores = 2 per core.

Math (per sample), exact up to quantization:
  small: y = As8 @ xs8 (fp8 DoubleRow, A scaled by 256)
         t16 = (y + us') * xse16          (us' = (u - A^T e_s)*256)
         score = sum_c t16 / 256          (fp16 ones-matmul reduce)
         exp fused into the PSUM exit (Act), row-sums/recip on DVE
         out_psum = Wc16 @ (attn*xse16) + I @ up(fm) ; out = fp16(psum + b_out)
  main:  same quadratic-form score at 32x32 (Am fp8-DR over 512 ch),
         fm = Wcm16 @ (attn_m*xme16)  (0.5 folded into Wcm host-side),
         fm w-duplicated into fm2 [128,2048] for the upsample matmul.
  Folds: W_align into main weights; attn*cc terms into inputs via
         e = solve(Wc, cc); proj+fuse into Wc; u adjusted by A^T e.
"""
import sys
sys.path.insert(0, '/opt/trn_rl_repo')
import numpy as np
import ml_dtypes

B, CS, CM = 16, 256, 512
HS = WS = 64
HM = WM = 32
NPX_S = HS * WS          # 4096
NPX_M = HM * WM          # 1024
NCORES = 8
BPC = B // NCORES        # 2
SCL = 256.0
F8 = ml_dtypes.float8_e4m3
NH = 2                   # small-branch halves per sample
HPX = NPX_S // NH        # 2048

_prog = None


def _build_program():
    import concourse.bacc as bacc
    import concourse.mybir as mybir
    from concourse.tile import TileContext

    f32, f16, f8 = mybir.dt.float32, mybir.dt.float16, mybir.dt.float8e4
    Act = mybir.ActivationFunctionType
    Alu = mybir.AluOpType
    DR = mybir.MatmulPerfMode.DoubleRow

    nc = bacc.Bacc(None, target_bir_lowering=False)

    xs8_d = nc.dram_tensor("xs8", [BPC, 128, 2, NPX_S], f8, kind="ExternalInput")
    xse_d = nc.dram_tensor("xse", [BPC, 2, 128, NPX_S], f16, kind="ExternalInput")
    xm8_d = nc.dram_tensor("xm8", [BPC, 128, 4, NPX_M], f8, kind="ExternalInput")
    xme_d = nc.dram_tensor("xme", [BPC, 128, 4, NPX_M], f16, kind="ExternalInput")
    As8_d = nc.dram_tensor("As8", [128, 2, 256], f8, kind="ExternalInput")
    Am8_d = nc.dram_tensor("Am8", [128, 4, 512], f8, kind="ExternalInput")
    WcT_d = nc.dram_tensor("WcT", [128, 2, 256], f16, kind="ExternalInput")
    WmT_d = nc.dram_tensor("WmT", [128, 4, 256], f16, kind="ExternalInput")
    I16_d = nc.dram_tensor("I16", [128, 128], f16, kind="ExternalInput")
    us_d = nc.dram_tensor("usv", [128, 2], f32, kind="ExternalInput")
    um_d = nc.dram_tensor("umv", [128, 4], f32, kind="ExternalInput")
    bo_d = nc.dram_tensor("bov", [128, 2], f32, kind="ExternalInput")
    out_d = nc.dram_tensor("out", [BPC, 2, 128, NPX_S], f16, kind="ExternalOutput")

    with TileContext(nc) as tc:
        with (
            tc.tile_pool(name="wp", bufs=1) as wp,
            tc.tile_pool(name="xp", bufs=1) as xp,
            tc.tile_pool(name="tp", bufs=1) as tp,
            tc.tile_pool(name="ps_y", bufs=2, space="PSUM") as ps_y,
            tc.tile_pool(name="ps_s", bufs=1, space="PSUM") as ps_s,
            tc.tile_pool(name="ps_v", bufs=2, space="PSUM") as ps_v,
        ):
            st = {}

            def dma_w(name, dram, shape, dtype):
                t = wp.tile(shape, dtype, tag=name, name=name)
                nc.sync.dma_start(out=t[:], in_=dram[:])
                return t

            # DMA emission order = availability order for the pipeline
            Am8 = dma_w("Am8", Am8_d, [128, 4, 512], f8)
            umv = dma_w("umv", um_d, [128, 4], f32)
            xm8, xme = [None] * BPC, [None] * BPC

            def load_xm(b):
                t = xp.tile([128, 4, NPX_M], f8, tag=f"xm8_{b}", name=f"xm8_{b}")
                nc.sync.dma_start(out=t[:], in_=xm8_d[b])
                xm8[b] = t
                tk = xp.tile([128, 4, NPX_M], f16, tag=f"xme_{b}", name=f"xme_{b}")
                nc.sync.dma_start(out=tk[:], in_=xme_d[b])
                xme[b] = tk

            xs8, xse = [None] * BPC, [None] * BPC

            def alloc_xs(b):
                t = xp.tile([128, 2, NPX_S], f8, tag=f"xs8_{b}", name=f"xs8_{b}")
                xs8[b] = t
                blocks = []
                for k in range(2):
                    tk = xp.tile([128, NPX_S], f16, tag=f"xse_{b}_{k}", name=f"xse_{b}_{k}")
                    blocks.append(tk)
                xse[b] = blocks

            def load_xs_half(b, h):
                nc.sync.dma_start(out=xs8[b][:, :, h * HPX:(h + 1) * HPX],
                                  in_=xs8_d[b, :, :, h * HPX:(h + 1) * HPX])
                for k in range(2):
                    nc.sync.dma_start(out=xse[b][k][:, h * HPX:(h + 1) * HPX],
                                      in_=xse_d[b, k, :, h * HPX:(h + 1) * HPX])

            alloc_xs(0)
            alloc_xs(1)
            load_xm(0)
            As8 = dma_w("As8", As8_d, [128, 2, 256], f8)
            usv = dma_w("usv", us_d, [128, 2], f32)
            load_xs_half(0, 0)
            load_xs_half(0, 1)
            WmT = dma_w("WmT", WmT_d, [128, 4, 256], f16)
            WcT = dma_w("WcT", WcT_d, [128, 2, 256], f16)
            I16 = dma_w("I16", I16_d, [128, 128], f16)
            bov = dma_w("bov", bo_d, [128, 2], f32)
            load_xs_half(1, 0)
            load_xm(1)
            load_xs_half(1, 1)

            ones16 = wp.tile([128, 1], f16, tag="ones16")
            nc.vector.memset(ones16[:], 1.0)

            # ---------------- unit emitters ----------------
            def ms_gemm(b):
                tm16 = tp.tile([128, 4, NPX_M], f16, tag="tm16", bufs=2, name=f"tm16_{b}")
                for m in range(4):
                    ym = ps_y.tile([128, NPX_M], f32, tag="y1k")
                    for nj in range(2):
                        for kk in range(2):
                            nc.tensor.matmul(
                                ym[:, nj * 512:(nj + 1) * 512],
                                Am8[:, 2 * kk:2 * kk + 2, m * 128:(m + 1) * 128],
                                xm8[b][:, 2 * kk:2 * kk + 2, nj * 512:(nj + 1) * 512],
                                start=(kk == 0), stop=(kk == 1), perf_mode=DR)
                    nc.vector.scalar_tensor_tensor(
                        tm16[:, m, :], ym[:], umv[:, m:m + 1], xme[b][:, m, :],
                        op0=Alu.add, op1=Alu.mult)
                em = tp.tile([32, 32], f16, tag="em32", bufs=2, name=f"em32_{b}")
                sps = ps_s.tile([1, 1024], f32, tag="s")
                for nj in range(2):
                    for k in range(4):
                        nc.tensor.matmul(sps[:, nj * 512:(nj + 1) * 512], ones16[:],
                                         tm16[:, k, nj * 512:(nj + 1) * 512],
                                         start=(k == 0), stop=(k == 3))
                ef = tp.tile([1, 1024], f16, tag="efm", bufs=2)
                nc.scalar.activation(ef[:], sps[:], Act.Exp, scale=1.0 / SCL)
                nc.sync.dma_start(out=em[:], in_=ef[:])
                st[("em", b)] = em

            def ms_tail(b, eng="dve"):
                em = st[("em", b)]
                esum = tp.tile([32, 1], f32, tag="esm", bufs=2)
                nc.vector.tensor_reduce(esum[:], em[:], axis=mybir.AxisListType.X,
                                        op=Alu.add)
                rec = tp.tile([32, 1], f32, tag="recm", bufs=2)
                nc.vector.reciprocal(rec[:], esum[:])
                at16 = tp.tile([32, 32], f16, tag="atm", bufs=2)
                nc.vector.tensor_scalar_mul(at16[:], em[:], rec[:])
                amf = tp.tile([1, NPX_M], f16, tag="amf", bufs=2)
                nc.sync.dma_start(out=amf[:], in_=at16[:])
                abm = tp.tile([128, NPX_M], f16, tag="abm", bufs=2)
                nc.gpsimd.partition_broadcast(abm[:], amf[:])
                axm = tp.tile([128, 4, NPX_M], f16, tag="axm", bufs=2, name=f"axm_{b}")
                engf = nc.vector if eng == "dve" else nc.gpsimd
                for k in range(4):
                    engf.tensor_tensor(axm[:, k, :], abm[:], xme[b][:, k, :],
                                       op=Alu.mult)
                st[("axm", b)] = axm

            def mv(b):
                axm = st[("axm", b)]
                fm_b = []
                for m in range(2):
                    fmt = tp.tile([128, 1024], f16, tag=f"fm_{b}_{m}", bufs=1,
                                  name=f"fm_{b}_{m}")
                    for nj in range(2):
                        fv = ps_v.tile([128, 512], f32, tag="v")
                        for k in range(4):
                            nc.tensor.matmul(fv[:],
                                             WmT[:, k, m * 128:(m + 1) * 128],
                                             axm[:, k, nj * 512:(nj + 1) * 512],
                                             start=(k == 0), stop=(k == 3))
                        nc.scalar.activation(fmt[:, nj * 512:(nj + 1) * 512],
                                             fv[:], Act.Identity)
                    fm_b.append(fmt)
                st[("fm2", b)] = fm_b

            def ss_gemm(b, h):
                for c in range(2):
                    cc = h * 2 + c
                    sc = tp.tile([16, 64], f16, tag="sc64", bufs=4)
                    px0 = cc * 1024
                    t16h = tp.tile([128, 2, 1024], f16, tag="t16h", bufs=3)
                    for m in range(2):
                        yps = ps_y.tile([128, 1024], f32, tag="y1k")
                        for nj in range(2):
                            nc.tensor.matmul(
                                yps[:, nj * 512:(nj + 1) * 512],
                                As8[:, :, m * 128:(m + 1) * 128],
                                xs8[b][:, :, px0 + nj * 512:px0 + (nj + 1) * 512],
                                start=True, stop=True, perf_mode=DR)
                        nc.vector.scalar_tensor_tensor(
                            t16h[:, m, :], yps[:], usv[:, m:m + 1],
                            xse[b][m][:, px0:px0 + 1024],
                            op0=Alu.add, op1=Alu.mult)
                    sps = ps_s.tile([1, 1024], f32, tag="s")
                    for nj in range(2):
                        for k in range(2):
                            nc.tensor.matmul(sps[:, nj * 512:(nj + 1) * 512], ones16[:],
                                             t16h[:, k, nj * 512:(nj + 1) * 512],
                                             start=(k == 0), stop=(k == 1))
                    ef = tp.tile([1, 1024], f16, tag="efs", bufs=3)
                    nc.scalar.activation(ef[:], sps[:], Act.Exp, scale=1.0 / SCL)
                    nc.sync.dma_start(out=sc[:], in_=ef[:])
                    st[("sc", b, cc)] = sc

            def ss_tail(b, h, eng="dve"):
                engf = nc.vector if eng == "dve" else nc.gpsimd
                for c in range(2):
                    cc = h * 2 + c
                    px0 = cc * 1024
                    sc = st[("sc", b, cc)]
                    esum = tp.tile([16, 1], f32, tag="ess", bufs=3)
                    nc.vector.tensor_reduce(esum[:], sc[:], axis=mybir.AxisListType.X,
                                            op=Alu.add)
                    rec = tp.tile([16, 1], f32, tag="recs", bufs=3)
                    nc.vector.reciprocal(rec[:], esum[:])
                    at16 = tp.tile([16, 64], f16, tag="ats", bufs=3)
                    nc.vector.tensor_scalar_mul(at16[:], sc[:], rec[:])
                    af = tp.tile([1, 1024], f16, tag="afs", bufs=3)
                    nc.sync.dma_start(out=af[:], in_=at16[:])
                    ab = tp.tile([128, 1024], f16, tag="abs", bufs=3)
                    nc.gpsimd.partition_broadcast(ab[:], af[:])
                    axs = tp.tile([128, 2, 1024], f16, tag="axs", bufs=5)
                    for k in range(2):
                        engf.tensor_tensor(axs[:, k, :], ab[:],
                                           xse[b][k][:, px0:px0 + 1024],
                                           op=Alu.mult)
                    st[("axs", b, cc)] = axs

            def sv(b, h, out_eng="act"):
                fm_b = st[("fm2", b)]
                for m in range(2):
                    fmv = fm_b[m][:].rearrange("p (hh one w) -> p hh one w",
                                               one=1, w=32)
                    for nj in range(4):
                        px0 = h * HPX + nj * 512
                        axs = st[("axs", b, h * 2 + nj // 2)]
                        vps = ps_v.tile([128, 512], f32, tag="v")
                        for k in range(2):
                            nc.tensor.matmul(
                                vps[:], WcT[:, k, m * 128:(m + 1) * 128],
                                axs[:, k, (nj % 2) * 512:(nj % 2 + 1) * 512],
                                start=(k == 0), stop=False)
                        h0 = (h * 4 + nj) * 4
                        up_ap = fmv[:, h0:h0 + 4].broadcast_to([128, 4, 2, 32])
                        vpd = vps[:].rearrange("a (w d) -> a w d", d=2)
                        nc.tensor.matmul(vpd[:, :, 0], I16[:], up_ap,
                                         start=False, stop=False)
                        nc.tensor.matmul(vpd[:, :, 1], I16[:], up_ap,
                                         start=False, stop=True)
                        if nj == 0:
                            o16 = tp.tile([128, 2048], f16, tag="o16", bufs=4)
                        if out_eng == "act":
                            nc.scalar.activation(o16[:, nj * 512:(nj + 1) * 512],
                                                 vps[:], Act.Identity,
                                                 bias=bov[:, m:m + 1], scale=1.0)
                        else:
                            nc.vector.tensor_scalar_add(o16[:, nj * 512:(nj + 1) * 512],
                                                        vps[:], bov[:, m:m + 1])
                    nc.sync.dma_start(out=out_d[b, m][:, h * HPX:(h + 1) * HPX],
                                      in_=o16[:])

            # ---------------- pipelined schedule ----------------
            import os
            sched = os.environ.get("K2_SCHED", "E")
            if sched == "A":
                units = [("ss", 0, 0), ("ms", 0), ("sst", 0, 0), ("mst", 0),
                         ("ss", 0, 1), ("mv", 0), ("sst", 0, 1), ("sv", 0, 0),
                         ("ss", 1, 0), ("sv", 0, 1), ("ss", 1, 1), ("ms", 1),
                         ("sst", 1, 0), ("mst", 1), ("mv", 1), ("sv", 1, 0),
                         ("sst", 1, 1), ("sv", 1, 1, "dve")]
            elif sched == "B":
                units = [("ss", 0, 0), ("ms", 0), ("sst", 0, 0), ("mst", 0),
                         ("ss", 0, 1), ("mv", 0), ("sst", 0, 1), ("sv", 0, 0),
                         ("ss", 1, 0), ("sv", 0, 1), ("ss", 1, 1), ("ms", 1),
                         ("sst", 1, 0), ("mst", 1), ("mv", 1), ("sv", 1, 0),
                         ("sst", 1, 1), ("sv", 1, 1)]
            elif sched == "C":
                units = [("ss", 0, 0), ("ms", 0), ("sst", 0, 0), ("mst", 0),
                         ("mv", 0), ("ss", 0, 1), ("sst", 0, 1), ("sv", 0, 0),
                         ("ms", 1), ("ss", 1, 0), ("mst", 1), ("sv", 0, 1),
                         ("mv", 1), ("sst", 1, 0), ("ss", 1, 1), ("sv", 1, 0),
                         ("sst", 1, 1), ("sv", 1, 1, "dve")]
            elif sched == "D":
                units = [("ms", 0), ("ss", 0, 0), ("mst", 0), ("sst", 0, 0),
                         ("mv", 0), ("ss", 0, 1), ("sst", 0, 1), ("sv", 0, 0),
                         ("ms", 1), ("ss", 1, 0), ("mst", 1), ("sv", 0, 1),
                         ("mv", 1), ("sst", 1, 0), ("ss", 1, 1), ("sv", 1, 0),
                         ("sst", 1, 1), ("sv", 1, 1, "dve")]
            elif sched == "E":
                units = [("ms", 0), ("ss", 0, 0), ("mst", 0), ("sst", 0, 0),
                         ("mv", 0), ("ss", 0, 1), ("sst", 0, 1), ("sv", 0, 0),
                         ("ms", 1), ("ss", 1, 0), ("mst", 1), ("sv", 0, 1),
                         ("mv", 1), ("sst", 1, 0), ("ss", 1, 1), ("sv", 1, 0),
                         ("sst", 1, 1), ("sv", 1, 1)]
            elif sched == "F":
                units = [("ms", 0), ("ss", 0, 0), ("mst", 0), ("sst", 0, 0),
                         ("mv", 0), ("ss", 0, 1), ("sst", 0, 1), ("ms", 1),
                         ("sv", 0, 0), ("ss", 1, 0), ("mst", 1), ("sv", 0, 1),
                         ("mv", 1), ("sst", 1, 0), ("ss", 1, 1), ("sv", 1, 0),
                         ("sst", 1, 1), ("sv", 1, 1, "dve")]
            elif sched == "G":
                units = [("ms", 0), ("ss", 0, 0), ("mst", 0), ("sst", 0, 0),
                         ("mv", 0), ("ss", 0, 1), ("sst", 0, 1), ("sv", 0, 0),
                         ("ms", 1), ("ss", 1, 0), ("mst", 1), ("ss", 1, 1),
                         ("sv", 0, 1), ("mv", 1), ("sst", 1, 0), ("sv", 1, 0),
                         ("sst", 1, 1), ("sv", 1, 1, "dve")]
            elif sched == "H":
                units = [("ms", 0), ("ss", 0, 0), ("mst", 0), ("sst", 0, 0),
                         ("ss", 0, 1), ("mv", 0), ("sst", 0, 1), ("sv", 0, 0),
                         ("ss", 1, 0), ("ms", 1), ("sst", 1, 0), ("sv", 0, 1),
                         ("mst", 1), ("ss", 1, 1), ("mv", 1), ("sst", 1, 1),
                         ("sv", 1, 0), ("sv", 1, 1, "dve")]
            for u in units:
                if u[0] == "ss":
                    ss_gemm(u[1], u[2])
                elif u[0] == "sst":
                    ss_tail(u[1], u[2])
                elif u[0] == "ms":
                    ms_gemm(u[1])
                elif u[0] == "mst":
                    ms_tail(u[1])
                elif u[0] == "mv":
                    mv(u[1])
                elif u[0] == "sv":
                    sv(u[1], u[2], out_eng=(u[3] if len(u) > 3 else "act"))

    nc.compile()
    return nc


def _prep_weights(W_align, b_align, pos_embed_main, pos_embed_small,
                  W_qkv_s, b_qkv_s, W_proj_s, b_proj_s, rel_pos_s,
                  W_qkv_m, b_qkv_m, W_proj_m, b_proj_m, rel_pos_m,
                  W_fuse, b_fuse):
    d = np.float64
    g = dict(W_align=W_align, b_align=b_align, W_qkv_s=W_qkv_s, b_qkv_s=b_qkv_s,
             W_proj_s=W_proj_s, b_proj_s=b_proj_s, W_qkv_m=W_qkv_m, b_qkv_m=b_qkv_m,
             W_proj_m=W_proj_m, b_proj_m=b_proj_m, W_fuse=W_fuse, b_fuse=b_fuse)
    g = {k: np.asarray(v, d) for k, v in g.items()}
    pos_s = np.asarray(pos_embed_small, d).reshape(-1)
    pos_m = np.asarray(pos_embed_main, d).reshape(-1)
    rel_s = np.asarray(rel_pos_s, d).reshape(-1)
    rel_m = np.asarray(rel_pos_m, d).reshape(-1)
    scale = 1.0 / np.sqrt(np.float64(CS))

    Wq, Wk, Wv = g['W_qkv_s'][:CS], g['W_qkv_s'][CS:2 * CS], g['W_qkv_s'][2 * CS:]
    bq, bk, bv = g['b_qkv_s'][:CS], g['b_qkv_s'][CS:2 * CS], g['b_qkv_s'][2 * CS:]
    cq = Wq @ pos_s + bq + rel_s
    ck = Wk @ pos_s + bk + rel_s
    cv_s = Wv @ pos_s + bv
    A_s = (Wq.T @ Wk) * scale
    u_s = (Wk.T @ cq + Wq.T @ ck) * scale

    cbase = g['b_align'] + pos_m
    Wqm = g['W_qkv_m'][:CS] @ g['W_align']
    Wkm = g['W_qkv_m'][CS:2 * CS] @ g['W_align']
    Wvm = g['W_qkv_m'][2 * CS:] @ g['W_align']
    cqm = g['W_qkv_m'][:CS] @ cbase + g['b_qkv_m'][:CS] + rel_m
    ckm = g['W_qkv_m'][CS:2 * CS] @ cbase + g['b_qkv_m'][CS:2 * CS] + rel_m
    cv_m = g['W_qkv_m'][2 * CS:] @ cbase + g['b_qkv_m'][2 * CS:]
    A_m = (Wqm.T @ Wkm) * scale
    u_m = (Wkm.T @ cqm + Wqm.T @ ckm) * scale

    W_fs = g['W_fuse'][:, :CS] @ g['W_proj_s']
    W_fm = (g['W_fuse'][:, CS:] @ g['W_proj_m']) * 0.5   # upsampled-softmax halving
    b_out = g['b_fuse'] + g['W_fuse'][:, :CS] @ g['b_proj_s'] \
        + g['W_fuse'][:, CS:] @ g['b_proj_m']
    Wc_s = W_fs @ Wv
    cc_s = W_fs @ cv_s
    Wc_m = W_fm @ Wvm
    cc_m = W_fm @ cv_m
    e_s = np.linalg.solve(Wc_s, cc_s)
    e_m = np.linalg.lstsq(Wc_m, cc_m, rcond=None)[0]
    us_adj = (u_s - (A_s.T @ e_s)) * SCL
    um_adj = (u_m - (A_m.T @ e_m)) * SCL

    f = np.float32

    def q8(x):
        return np.clip(np.asarray(x, f), -240, 240).astype(F8)

    def colvec(v, chunks):
        return np.ascontiguousarray(np.asarray(v, f).reshape(chunks, 128).T)

    As8 = q8((A_s * SCL).T.reshape(2, 128, 256).transpose(1, 0, 2))
    Am8 = q8((A_m * SCL).T.reshape(4, 128, 512).transpose(1, 0, 2))
    WcT = np.ascontiguousarray(Wc_s.T.reshape(2, 128, 256).transpose(1, 0, 2)
                               .astype(np.float16))
    WmT = np.ascontiguousarray(Wc_m.T.reshape(4, 128, 256).transpose(1, 0, 2)
                               .astype(np.float16))
    return {
        "As8": np.ascontiguousarray(As8),
        "Am8": np.ascontiguousarray(Am8),
        "WcT": WcT, "WmT": WmT,
        "I16": np.eye(128, dtype=np.float16),
        "usv": colvec(us_adj, 2), "umv": colvec(um_adj, 4),
        "bov": colvec(b_out, 2),
    }, e_s.astype(f), e_m.astype(f)


def kernel(**inputs):
    global _prog
    from concourse.bass_utils import run_bass_kernel_spmd

    small = np.ascontiguousarray(np.asarray(inputs["small_feat"], dtype=np.float32))
    main = np.ascontiguousarray(np.asarray(inputs["main_feat"], dtype=np.float32))
    w, e_s, e_m = _prep_weights(**{k: np.asarray(v) for k, v in inputs.items()
                                   if k not in ("small_feat", "main_feat")})

    xs = small.reshape(B, CS, NPX_S)
    xm = main.reshape(B, CM, NPX_M)
    xs8 = np.clip(xs, -240, 240).astype(F8).reshape(B, 2, 128, NPX_S).transpose(0, 2, 1, 3)
    xse = (xs + e_s[None, :, None]).astype(np.float16).reshape(B, 2, 128, NPX_S)
    xm8 = np.clip(xm, -240, 240).astype(F8).reshape(B, 4, 128, NPX_M).transpose(0, 2, 1, 3)
    xme = (xm + e_m[None, :, None]).astype(np.float16).reshape(B, 4, 128, NPX_M) \
        .transpose(0, 2, 1, 3)

    if _prog is None:
        _prog = _build_program()
    nc = _prog

    in_maps = []
    for c in range(NCORES):
        m = dict(w)
        sl = slice(c * BPC, (c + 1) * BPC)
        m["xs8"] = np.ascontiguousarray(xs8[sl])
        m["xse"] = np.ascontiguousarray(xse[sl])
        m["xm8"] = np.ascontiguousarray(xm8[sl])
        m["xme"] = np.ascontiguousarray(xme[sl])

        in_maps.append(m)

    res = run_bass_kernel_spmd(nc, in_maps, list(range(NCORES)))
    out = np.concatenate([r["out"] for r in res.results], axis=0)
    return out.astype(np.float32).reshape(B, CS, HS, WS)
